# revision 1
# baseline (speedup 1.0000x reference)
"""Trainium2 Bass kernel for nn_BCTransformer: B=131072 batch of tiny 2-token
4-layer transformer encoder forward passes.

Strategy: pure data parallel over 8 NeuronCores (16384 batch each).  Within a
core, activations live feature-major [D=128 partitions, columns], columns =
(token, batch).  The whole network is fused in SBUF per super-tile of 1024
columns (512 batch x 2 tokens); 32 super-tiles per core.

Key tricks:
 - matmuls run in float32r (full PE rate, ~1e-4 rel err) via AP bitcast.
 - LayerNorm over the partition dim: centering matrix (I - J/128) as one
   matmul, variance via (J/256) matmul on Square(hc), rstd via a custom DVE
   op (bit-trick seed + 2 Newton iterations).  No ACT table needed.
 - softmax over S=2 tokens == sigmoid == 0.5 + 0.5*tanh(d/2); attention out
   o_q = (v0+v1)/2 + 0.5*t_q*(v0-v1), with the 0.5s folded into W_out.
 - Exact gelu / tanh / square all live in one ACT table set (gelu_and_others)
   => zero table switches.
 - Linear biases + LN affine folds ride psum->sbuf copies or are folded into
   the next layer's weights host-side.  Residual adds are identity matmuls
   accumulated into PSUM.
"""
import sys

sys.path.insert(0, "/opt/trn_rl_repo")

import math
from contextlib import ExitStack

import numpy as np

import concourse.bass as bass
import concourse.tile as tile
from concourse import bacc, mybir
from concourse.bass_utils import run_bass_kernel_spmd

# ---------------------------------------------------------------- constants
D = 128
NH = 4
HD = 32
FF = 256
L = 4
S = 2
B = 131072
EPS = 1e-5
NCORES = 8
BP = B // NCORES          # batch per core = 16384
N = 512                   # batch elems per super-tile
NT = 2 * N                # columns per super-tile (tok0 block | tok1 block)
NTILES = BP // N          # 32
MMC = 512                 # matmul column chunk

F32 = mybir.dt.float32
F32R = mybir.dt.float32r
I32 = mybir.dt.int32
AF = mybir.ActivationFunctionType
ALU = mybir.AluOpType

# ------------------------------------------------- custom DVE rsqrt op
MAGIC = 0x5F375A86
MAGIC_VH = MAGIC - (1 << 22)   # seed computed from bits of vh = v/2
SEED_ADD = MAGIC_VH + 1        # seed = ~(i_vh >> 1) + SEED_ADD


def _register_rsqrt_op():
    import concourse.dve_ops as dve_ops
    from concourse.dve_ops import DveOp
    from concourse.dve_spec import C0, Spec, Src0, Src1, lower, _has_src1
    from concourse.dve_uop import DveOpSpec

    name = "RSQRT_NR2_ANT"
    if name in dve_ops._SUB_OPCODE_FOR_NAME:
        for op in dve_ops.OPS:
            if op.name == name:
                return op

    def _ref(in0, in1, c0, c1, c2):
        vh = in0.astype(np.float32)
        s = in1.astype(np.float32)
        y1 = s * (c0 - vh * s * s)
        y2 = y1 * (c0 - vh * y1 * y1)
        return y2.astype(np.float32)

    _y1 = Src1 * (C0 - Src0 * (Src1 * Src1))
    spec = Spec(body=_y1 * (C0 - Src0 * (_y1 * _y1)), reference=_ref)
    opcode = dve_ops._CUSTOM_DVE_ROW_BASE + len(dve_ops.OPS)
    assert opcode < 0x20
    dve_ops._SUB_OPCODE_FOR_NAME[name] = opcode
    shas = {}
    for ver in ("v3", "v4"):
        try:
            uops = lower(spec, ver=ver)
            shas[ver] = DveOpSpec(
                name=name, opcode=opcode, uops=uops, rd1_en=_has_src1(spec)
            ).sha(ver)
        except Exception:
            pass
    op = DveOp(name, spec, subdim=False, uops_sha=shas)
    dve_ops.OPS.append(op)
    dve_ops.CUSTOM_DVE_SPECS[name] = spec
    return op


RSQRT_NR2 = _register_rsqrt_op()


# ------------------------------------------------- host-side weight folding
def _prepare_weights(p):
    f = lambda a: np.asarray(a, np.float32)
    out = {}
    out["w_inT"] = np.ascontiguousarray((f(p["w_in"]) * math.sqrt(D)).T)
    b_in = (f(p["b_in"]) * math.sqrt(D))[:, None]
    pos = np.arange(10, dtype=np.float32)[:, None]
    div = np.exp(np.arange(0, D, 2, dtype=np.float32) * (-math.log(10000.0) / D))
    pe = np.zeros((10, D), dtype=np.float32)
    pe[:, 0::2] = np.sin(pos * div)
    pe[:, 1::2] = np.cos(pos * div)
    g_in = f(p["g_in"])[:, None]
    bias_e0 = (f(p["bt_in"]) + pe[0])[:, None]
    bias_e1 = (f(p["bt_in"]) + pe[1])[:, None]
    out["eb"] = np.ascontiguousarray(
        np.concatenate([b_in, g_in, bias_e0, bias_e1], axis=1))  # [128,4]

    wl = np.zeros((L, 128, 1024), np.float32)
    blb = np.zeros((L, 128, 8), np.float32)
    for l in range(L):
        g1 = f(p["n1_g"][l]); b1 = f(p["n1_b"][l])
        qkv_w = f(p["qkv_w"][l]); qkv_b = f(p["qkv_b"][l])
        wqkvT = (qkv_w * g1[None, :]).T          # [128, 384]
        bqkv = qkv_b + qkv_w @ b1
        out_w = f(p["out_w"][l])
        woT_half = (0.5 * out_w).T               # [128,128]
        g2 = f(p["n2_g"][l]); b2 = f(p["n2_b"][l])
        ff1_w = f(p["ff1_w"][l]); ff1_b = f(p["ff1_b"][l])
        ff1T = (ff1_w * g2[None, :]).T           # [128, 256]
        bff1 = ff1_b + ff1_w @ b2
        ff2T = f(p["ff2_w"][l]).T                # [256, 128]
        wl[l, :, 0:384] = wqkvT
        wl[l, :, 384:512] = woT_half
        wl[l, :, 512:768] = ff1T
        wl[l, :, 768:896] = ff2T[0:128]
        wl[l, :, 896:1024] = ff2T[128:256]
        blb[l, :, 0] = bqkv[0:128]
        blb[l, :, 1] = bqkv[128:256]
        blb[l, :, 2] = bqkv[256:384]
        blb[l, :, 3] = f(p["out_b"][l])
        blb[l, :, 4] = bff1[0:128]
        blb[l, :, 5] = bff1[128:256]
        blb[l, :, 6] = f(p["ff2_b"][l])
    out["wl"] = wl
    out["bl"] = blb

    go = f(p["g_out"]); bo = f(p["bt_out"])
    h1_w = f(p["h1_w"])
    wh = np.zeros((128, 193), np.float32)
    wh[:, 0:128] = (0.5 * h1_w * go[None, :]).T
    wh[:, 128:192] = f(p["h2_w"]).T
    wh[0:64, 192] = f(p["h3_w"])[0]
    out["wh"] = wh
    bh = np.zeros((128, 3), np.float32)
    bh[:, 0] = f(p["h1_b"]) + h1_w @ bo
    bh[0:64, 1] = f(p["h2_b"])
    bh[0, 2] = f(p["h3_b"])[0]
    out["bh"] = bh
    return out


def _static_consts():
    c = {}
    c["C"] = (np.eye(128, dtype=np.float32) - 1.0 / 128.0).astype(np.float32)
    c["Jv"] = np.full((128, 128), 1.0 / 256.0, np.float32)
    c["I"] = np.eye(128, dtype=np.float32)
    c["epsrow"] = np.full((1, 128), EPS / 2.0, np.float32)
    sm = np.zeros((128, 4), np.float32)
    for d in range(128):
        sm[d, d // HD] = 1.0 / math.sqrt(HD)
    c["smask"] = sm
    bc = np.zeros((36, 256), np.float32)
    for d in range(128):
        bc[0 + d // HD, 0 * 128 + d] = 1.0
        bc[32 + d // HD, 1 * 128 + d] = 1.0
    c["bcmask"] = bc
    return c


def r32(ap):
    return ap.bitcast(F32R)


def _mm(nc, out_ps, lhsT, rhs, start, stop):
    """float32r matmul, chunked over the free dim (<=MMC cols per call)."""
    n = rhs.shape[-1]
    nch = (n + MMC - 1) // MMC
    for c in range(nch):
        sl = slice(c * MMC, min((c + 1) * MMC, n))
        nc.tensor.matmul(out_ps[:, sl], lhsT, rhs[:, sl],
                         start=start, stop=stop)


def build_nc(ntiles=NTILES):
    nc = bacc.Bacc(None, target_bir_lowering=False)
    cst = _static_consts()

    x_d = nc.dram_tensor("x", [BP, 4], F32, kind="ExternalInput")
    wl_d = nc.dram_tensor("wl", [L, 128, 1024], F32, kind="ExternalInput")
    bl_d = nc.dram_tensor("bl", [L, 128, 8], F32, kind="ExternalInput")
    winT_d = nc.dram_tensor("w_inT", [2, 128], F32, kind="ExternalInput")
    eb_d = nc.dram_tensor("eb", [128, 4], F32, kind="ExternalInput")
    wh_d = nc.dram_tensor("wh", [128, 193], F32, kind="ExternalInput")
    bh_d = nc.dram_tensor("bh", [128, 3], F32, kind="ExternalInput")
    o_d = nc.dram_tensor("o", [1, BP], F32, kind="ExternalOutput")

    C_d = nc.inline_tensor(cst["C"], name="Cmat")
    Jv_d = nc.inline_tensor(cst["Jv"], name="Jvmat")
    I_d = nc.inline_tensor(cst["I"], name="Imat")
    eps_d = nc.inline_tensor(cst["epsrow"], name="epsrow")
    sm_d = nc.inline_tensor(cst["smask"], name="smask")
    bc_d = nc.inline_tensor(cst["bcmask"], name="bcmask")

    with tile.TileContext(nc) as tc, ExitStack() as ctx:
        wp = ctx.enter_context(tc.tile_pool(name="weights", bufs=1))
        hp = ctx.enter_context(tc.tile_pool(name="hbuf", bufs=3))
        sp = ctx.enter_context(tc.tile_pool(name="scratch", bufs=2))
        qp = ctx.enter_context(tc.tile_pool(name="qkv", bufs=3))
        psA = ctx.enter_context(tc.tile_pool(name="psA", bufs=2, space="PSUM"))
        psB = ctx.enter_context(tc.tile_pool(name="psB", bufs=4, space="PSUM"))

        def wtile(src, shape, tag):
            t = wp.tile(shape, F32, tag=tag)
            nc.sync.dma_start(t[:], src)
            return t

        def wtile_r(src, shape, tag):
            st = sp.tile([128, 1024], F32, tag="wstage")
            sv = st[: shape[0], : shape[1]]
            nc.sync.dma_start(sv, src)
            t = wp.tile(shape, F32R, tag=tag)
            nc.scalar.copy(t[:], sv)
            return t

        wl_t = [wtile_r(wl_d[l], [128, 1024], f"wl{l}") for l in range(L)]
        bl_t = [wtile(bl_d[l], [128, 8], f"bl{l}") for l in range(L)]
        winT_t = wtile_r(winT_d[:], [2, 128], "winT")
        eb_t = wtile(eb_d[:], [128, 4], "eb")
        wh_t = wtile_r(wh_d[:], [128, 193], "wh")
        bh_t = wtile(bh_d[:], [128, 3], "bh")
        C_t = wtile_r(C_d[:], [128, 128], "Cm")
        Jv_t = wtile_r(Jv_d[:], [128, 128], "Jv")
        I_t = wtile_r(I_d[:], [128, 128], "Im")
        eps_t = wtile_r(eps_d[:], [1, 128], "epsr")
        sm_t = wtile_r(sm_d[:], [128, 4], "smask")
        sm_bb = wp.tile([128, 4], mybir.dt.bfloat16, tag="smaskb")
        nc.vector.tensor_copy(sm_bb[:], sm_t[:].bitcast(F32))
        bc_t = wtile_r(bc_d[:], [36, 256], "bcm")
        ones_s = sp.tile([128, 1024], F32, tag="wstage")
        nc.vector.memset(ones_s[0:1, 0:NT], 1.0)
        ones_t = wp.tile([1, NT], F32R, tag="ones")
        nc.scalar.copy(ones_t[:], ones_s[0:1, 0:NT])

        b_in_ap = eb_t[:, 0:1]
        g_in_ap = eb_t[:, 1:2]
        bias_e0 = eb_t[:, 2:3]
        bias_e1 = eb_t[:, 3:4]

        def layernorm(hsb, tag="", y_dt=F32R):
            """y = (h - mean)/sqrt(var + eps), [128, NT] sbuf tile.
            Pipelined in two column halves (1-bank psum tiles) so the
            serial square->var->seed->rsqrt->apply chain overlaps."""
            y = sp.tile([128, NT], y_dt, tag="yln")
            for hf in range(2):
                cs = slice(hf * N, (hf + 1) * N)
                hc_ps = psB.tile([128, N], F32, tag="lnh")
                nc.tensor.matmul(hc_ps[:], C_t[:], hsb[:, cs],
                                 start=True, stop=True)
                sq = sp.tile([128, N], F32R, tag="sq")
                nc.scalar.activation(out=sq[:], in_=hc_ps[:], func=AF.Square,
                                     bias=0.0, scale=1.0)
                vh_ps = psB.tile([128, N], F32, tag="lnh")
                nc.tensor.matmul(vh_ps[:], eps_t[:], ones_t[:, 0:N],
                                 start=True, stop=False)
                nc.tensor.matmul(vh_ps[:], Jv_t[:], sq[:],
                                 start=False, stop=True)
                tb = sp.tile([128, N], I32, tag="tbits")
                nc.vector.tensor_scalar(
                    tb[:], vh_ps[:].bitcast(I32), 1, 0xFFFFFFFF,
                    op0=ALU.logical_shift_right, op1=ALU.bitwise_xor)
                nc.gpsimd.tensor_scalar(tb[:], tb[:], SEED_ADD, None,
                                         op0=ALU.add)
                R = sp.tile([128, N], F32, tag="rstd")
                nc.vector._custom_dve(RSQRT_NR2, out=R[:], in0=vh_ps[:],
                                      in1=tb[:].bitcast(F32), s0=1.5)
                nc.vector.tensor_mul(y[:, cs], hc_ps[:], R[:])
            return y

        def emit_embed(it):
            b0 = it * N
            xs0 = sp.tile([2, N], F32, tag="xs0")
            xs1 = sp.tile([2, N], F32, tag="xs1")
            xs = x_d[b0:b0 + N, :]
            nc.sync.dma_start(xs0[:], xs.rearrange("n f -> f n")[0:2, :])
            nc.sync.dma_start(xs1[:], xs.rearrange("n f -> f n")[2:4, :])
            xt0 = sp.tile([2, N], F32R, tag="xt0")
            nc.scalar.copy(xt0[:], xs0[:])
            xt1 = sp.tile([2, N], F32R, tag="xt1")
            nc.scalar.copy(xt1[:], xs1[:])

            pe0 = psA.tile([128, NT], F32, tag="psbig")
            _mm(nc, pe0[:, 0:N], winT_t[:], xt0[:], start=True, stop=True)
            _mm(nc, pe0[:, N:NT], winT_t[:], xt1[:], start=True, stop=True)
            h_emb = sp.tile([128, NT], F32R, tag="hemb")
            nc.scalar.activation(out=h_emb[:], in_=pe0[:], func=AF.Identity,
                                 bias=b_in_ap, scale=1.0)

            y_e = layernorm(h_emb, tag="e", y_dt=F32)
            h = hp.tile([128, NT], F32R, tag="h")
            nc.vector.tensor_scalar(h[:, 0:N], y_e[:, 0:N], g_in_ap, bias_e0,
                                    op0=ALU.mult, op1=ALU.add)
            nc.vector.tensor_scalar(h[:, N:NT], y_e[:, N:NT], g_in_ap, bias_e1,
                                    op0=ALU.mult, op1=ALU.add)
            return h

        def emit_layer(l, h):
            W = wl_t[l]
            Bb = bl_t[l]
            y1 = layernorm(h, tag="1")
            qkv_sb = []
            for j in range(3):
                ps = psA.tile([128, NT], F32, tag="psbig")
                _mm(nc, ps, W[:, 128 * j:128 * (j + 1)], y1[:],
                    start=True, stop=True)
                dt_j = mybir.dt.bfloat16 if j < 2 else F32R
                t = qp.tile([128, NT], dt_j, tag=f"qkv{j}")
                if j == 0:
                    nc.vector.tensor_scalar(t[:], ps[:], Bb[:, j:j + 1], None,
                                            op0=ALU.add)
                else:
                    nc.scalar.activation(out=t[:], in_=ps[:], func=AF.Identity,
                                         bias=Bb[:, j:j + 1], scale=1.0)
                qkv_sb.append(t)
            q_sb, k_sb, v_sb = qkv_sb
            dk = sp.tile([128, N], mybir.dt.bfloat16, tag="dk")
            nc.gpsimd.tensor_tensor(dk[:], k_sb[:, 0:N], k_sb[:, N:NT],
                                    op=ALU.subtract)
            pr = sp.tile([128, 2, N], mybir.dt.bfloat16, tag="prods")
            apk = dk[:]
            dk_b = bass.AP(tensor=apk.tensor, offset=apk.offset,
                           ap=[apk.ap[0], [0, 2], apk.ap[1]])
            nc.vector.tensor_mul(
                pr[:], q_sb[:].rearrange("p (q n) -> p q n", q=2), dk_b)
            d_ps = psB.tile([36, N], F32, tag="lnh")
            nc.tensor.matmul(d_ps[0:4, :], sm_bb[:], pr[:, 0, :],
                             start=True, stop=True)
            nc.tensor.matmul(d_ps[32:36, :], sm_bb[:], pr[:, 1, :],
                             start=True, stop=True, tile_position=(0, 32))
            T8 = sp.tile([36, N], F32R, tag="T8")
            nc.scalar.activation(out=T8[:], in_=d_ps[:],
                                 func=AF.Tanh, bias=0.0, scale=0.5)
            dv = sp.tile([128, N], mybir.dt.bfloat16, tag="dv")
            vf = v_sb[:].bitcast(F32)
            nc.gpsimd.tensor_tensor(dv[:], vf[:, 0:N], vf[:, N:NT],
                                    op=ALU.subtract)
            tb_ps = psA.tile([128, NT], F32, tag="psbig")
            nc.tensor.matmul(tb_ps[:, 0:N], bc_t[:, 0:128], T8[:],
                             start=True, stop=True)
            nc.tensor.matmul(tb_ps[:, N:NT], bc_t[:, 128:256], T8[:],
                             start=True, stop=True)
            u = sp.tile([128, NT], F32R, tag="u")
            ap0 = dv[:]
            dv_b = bass.AP(tensor=ap0.tensor, offset=ap0.offset,
                           ap=[ap0.ap[0], [0, 2], ap0.ap[1]])
            nc.vector.tensor_mul(
                u[:].rearrange("p (q n) -> p q n", q=2),
                tb_ps[:].rearrange("p (q n) -> p q n", q=2), dv_b)
            p1 = psA.tile([128, NT], F32, tag="psbig")
            woT = W[:, 384:512]
            _mm(nc, p1, I_t[:], h[:], start=True, stop=False)
            for qi in range(2):
                sl = slice(qi * N, (qi + 1) * N)
                nc.tensor.matmul(p1[:, sl], woT, v_sb[:, 0:N],
                                 start=False, stop=False)
                nc.tensor.matmul(p1[:, sl], woT, v_sb[:, N:NT],
                                 start=False, stop=False)
                nc.tensor.matmul(p1[:, sl], woT, u[:, sl],
                                 start=False, stop=True)
            h2t = hp.tile([128, NT], F32R, tag="h")
            nc.scalar.activation(out=h2t[:], in_=p1[:], func=AF.Identity,
                                 bias=Bb[:, 3:4], scale=1.0)
            h = h2t
            y2 = layernorm(h, tag="2")
            f0 = psA.tile([128, NT], F32, tag="psbig")
            _mm(nc, f0, W[:, 512:640], y2[:], start=True, stop=True)
            f1 = psA.tile([128, NT], F32, tag="psbig")
            _mm(nc, f1, W[:, 640:768], y2[:], start=True, stop=True)
            g0 = sp.tile([128, NT], F32R, tag="g0")
            nc.scalar.activation(out=g0[:], in_=f0[:], func=AF.Gelu,
                                 bias=Bb[:, 4:5], scale=1.0)
            g1 = sp.tile([128, NT], F32R, tag="g1")
            nc.scalar.activation(out=g1[:], in_=f1[:], func=AF.Gelu,
                                 bias=Bb[:, 5:6], scale=1.0)
            p2 = psA.tile([128, NT], F32, tag="psbig")
            _mm(nc, p2, I_t[:], h[:], start=True, stop=False)
            _mm(nc, p2, W[:, 768:896], g0[:], start=False, stop=False)
            _mm(nc, p2, W[:, 896:1024], g1[:], start=False, stop=True)
            h3t = hp.tile([128, NT], F32R, tag="h")
            nc.scalar.activation(out=h3t[:], in_=p2[:], func=AF.Identity,
                                 bias=Bb[:, 6:7], scale=1.0)
            return h3t

        def emit_head(it, h):
            b0 = it * N
            yf = layernorm(h, tag="f")
            p3 = psB.tile([128, N], F32, tag="lnh")
            nc.tensor.matmul(p3[:], wh_t[:, 0:128], yf[:, 0:N],
                             start=True, stop=False)
            nc.tensor.matmul(p3[:], wh_t[:, 0:128], yf[:, N:NT],
                             start=False, stop=True)
            p1h = sp.tile([128, N], F32R, tag="p1h")
            nc.scalar.activation(out=p1h[:], in_=p3[:], func=AF.Gelu,
                                 bias=bh_t[:, 0:1], scale=1.0)
            p4 = psB.tile([64, N], F32, tag="lnh")
            nc.tensor.matmul(p4[:], wh_t[:, 128:192], p1h[:],
                             start=True, stop=True)
            p2h = sp.tile([64, N], F32R, tag="p2h")
            nc.scalar.activation(out=p2h[:], in_=p4[:], func=AF.Gelu,
                                 bias=bh_t[0:64, 1:2], scale=1.0)
            p5 = psB.tile([1, N], F32, tag="lnh")
            nc.tensor.matmul(p5[:], wh_t[0:64, 192:193], p2h[:],
                             start=True, stop=True)
            th = sp.tile([1, N], F32, tag="th")
            nc.scalar.activation(out=th[:], in_=p5[:], func=AF.Tanh,
                                 bias=bh_t[0:1, 2:3], scale=1.0)
            res = sp.tile([1, N], F32, tag="res")
            nc.gpsimd.tensor_scalar(res[:], th[:], 3.0, None, op0=ALU.mult)
            nc.sync.dma_start(o_d[0:1, b0:b0 + N], res[:])

        # software-pipeline pairs of super-tiles: interleave emission so the
        # scheduler fills one tile's serial-chain stalls with the other's work
        pairs = ntiles // 2
        for p in range(pairs):
            ia, ib = 2 * p, 2 * p + 1
            ha = emit_embed(ia)
            hb = emit_embed(ib)
            for l in range(L):
                ha = emit_layer(l, ha)
                hb = emit_layer(l, hb)
            emit_head(ia, ha)
            emit_head(ib, hb)
        for it in range(pairs * 2, ntiles):
            h = emit_embed(it)
            for l in range(L):
                h = emit_layer(l, h)
            emit_head(it, h)

    nc.compile()
    return nc


_NC_CACHE = {}


def kernel(**inputs):
    w = _prepare_weights(inputs)
    if "nc" not in _NC_CACHE:
        _NC_CACHE["nc"] = build_nc()
    nc = _NC_CACHE["nc"]
    x = np.asarray(inputs["x"], np.float32)
    in_maps = []
    for c in range(NCORES):
        in_maps.append({
            "x": np.ascontiguousarray(x[c * BP:(c + 1) * BP]),
            "wl": w["wl"], "bl": w["bl"], "w_inT": w["w_inT"],
            "eb": w["eb"], "wh": w["wh"], "bh": w["bh"],
        })
    res = run_bass_kernel_spmd(nc, in_maps, core_ids=list(range(NCORES)))
    outs = [res.results[c]["o"].reshape(BP, 1) for c in range(NCORES)]
    return np.concatenate(outs, axis=0).astype(np.float32)


if __name__ == "__main__":
    build_nc(ntiles=1)
    print("build ok")



# revision 53
# speedup vs baseline: 122.2202x; 122.2202x over previous
"""Trainium2 Bass kernel for nn_BCTransformer: B=131072 batch of tiny 2-token
4-layer transformer encoder forward passes.

Strategy: pure data parallel over 8 NeuronCores (16384 batch each).  Within a
core, activations live feature-major [D=128 partitions, columns], columns =
(token, batch).  The whole network is fused in SBUF per super-tile of 1024
columns (512 batch x 2 tokens); 32 super-tiles per core.

Key tricks:
 - matmuls run in float32r (full PE rate, ~1e-4 rel err) via AP bitcast.
 - LayerNorm over the partition dim: centering matrix (I - J/128) as one
   matmul, variance via (J/256) matmul on Square(hc), rstd via a custom DVE
   op (bit-trick seed + 2 Newton iterations, all on DVE).  The eps seed
   matmul only exists for the embed LN (post-embed variances are >= 0.7).
 - softmax over S=2 tokens == sigmoid == 0.5 + 0.5*tanh(d/2); attention out
   o_q = (v0+v1)/2 + 0.5*t_q*(v0-v1), with the 0.5s folded into W_out;
   q is consumed straight from PSUM by the pr muls (no SBUF copy) and
   v0+v1 / v0-v1 are formed once on GpSimd.
 - Exact gelu / tanh / square all live in one ACT table set (gelu_and_others)
   => zero table switches.
 - Linear biases + LN affine folds ride psum->sbuf copies or are folded into
   the next layer's weights host-side.  Residual adds are identity matmuls
   accumulated into PSUM.
 - Emission is phase-rotated over groups of 3 super-tiles: each pipeline
   stage (LN1 / qkv / score / attn-out / LN2 / ff) is emitted for all
   chains before the next stage, so the in-order engine queues hide one
   chain's serial LN/attention latency behind the other chains' work;
   heads ride behind the next group's embeds.  PSUM runs as two 4-bank
   rings of [128,512] tiles.
"""
import sys

sys.path.insert(0, "/opt/trn_rl_repo")

import math
from contextlib import ExitStack

import numpy as np

import concourse.bass as bass
import concourse.tile as tile
from concourse import bacc, mybir
from concourse.bass_utils import run_bass_kernel_spmd

# ---------------------------------------------------------------- constants
D = 128
NH = 4
HD = 32
FF = 256
L = 4
S = 2
B = 131072
EPS = 1e-5
NCORES = 8
BP = B // NCORES          # batch per core = 16384
N = 512                   # batch elems per super-tile
NT = 2 * N                # columns per super-tile (tok0 block | tok1 block)
NTILES = BP // N          # 32
MMC = 512                 # matmul column chunk

F32 = mybir.dt.float32
F32R = mybir.dt.float32r
I32 = mybir.dt.int32
AF = mybir.ActivationFunctionType
ALU = mybir.AluOpType

# ------------------------------------------------- custom DVE rsqrt op
MAGIC = 0x5F375A86
MAGIC_VH = MAGIC - (1 << 22)   # seed computed from bits of vh = v/2
SEED_ADD = MAGIC_VH + 1        # seed = ~(i_vh >> 1) + SEED_ADD


def _register_rsqrt_op():
    import concourse.dve_ops as dve_ops
    from concourse.dve_ops import DveOp
    from concourse.dve_spec import C0, Spec, Src0, Src1, lower, _has_src1
    from concourse.dve_uop import DveOpSpec

    name = "RSQRT_NR2_ANT"
    if name in dve_ops._SUB_OPCODE_FOR_NAME:
        for op in dve_ops.OPS:
            if op.name == name:
                return op

    def _ref(in0, in1, c0, c1, c2):
        vh = in0.astype(np.float32)
        s = in1.astype(np.float32)
        y1 = s * (c0 - vh * s * s)
        y2 = y1 * (c0 - vh * y1 * y1)
        return y2.astype(np.float32)

    _y1 = Src1 * (C0 - Src0 * (Src1 * Src1))
    spec = Spec(body=_y1 * (C0 - Src0 * (_y1 * _y1)), reference=_ref)
    opcode = dve_ops._CUSTOM_DVE_ROW_BASE + len(dve_ops.OPS)
    assert opcode < 0x20
    dve_ops._SUB_OPCODE_FOR_NAME[name] = opcode
    shas = {}
    for ver in ("v3", "v4"):
        try:
            uops = lower(spec, ver=ver)
            shas[ver] = DveOpSpec(
                name=name, opcode=opcode, uops=uops, rd1_en=_has_src1(spec)
            ).sha(ver)
        except Exception:
            pass
    op = DveOp(name, spec, subdim=False, uops_sha=shas)
    dve_ops.OPS.append(op)
    dve_ops.CUSTOM_DVE_SPECS[name] = spec
    return op


RSQRT_NR2 = _register_rsqrt_op()


# ------------------------------------------------- host-side weight folding
def _prepare_weights(p):
    f = lambda a: np.asarray(a, np.float32)
    out = {}
    out["w_inT"] = np.ascontiguousarray((f(p["w_in"]) * math.sqrt(D)).T)
    b_in = (f(p["b_in"]) * math.sqrt(D))[:, None]
    pos = np.arange(10, dtype=np.float32)[:, None]
    div = np.exp(np.arange(0, D, 2, dtype=np.float32) * (-math.log(10000.0) / D))
    pe = np.zeros((10, D), dtype=np.float32)
    pe[:, 0::2] = np.sin(pos * div)
    pe[:, 1::2] = np.cos(pos * div)
    g_in = f(p["g_in"])[:, None]
    bias_e0 = (f(p["bt_in"]) + pe[0])[:, None]
    bias_e1 = (f(p["bt_in"]) + pe[1])[:, None]
    out["eb"] = np.ascontiguousarray(
        np.concatenate([b_in, g_in, bias_e0, bias_e1], axis=1))  # [128,4]

    wl = np.zeros((L, 128, 1024), np.float32)
    blb = np.zeros((L, 128, 8), np.float32)
    for l in range(L):
        g1 = f(p["n1_g"][l]); b1 = f(p["n1_b"][l])
        qkv_w = f(p["qkv_w"][l]); qkv_b = f(p["qkv_b"][l])
        wqkvT = (qkv_w * g1[None, :]).T          # [128, 384]
        bqkv = qkv_b + qkv_w @ b1
        out_w = f(p["out_w"][l])
        woT_half = (0.5 * out_w).T               # [128,128]
        g2 = f(p["n2_g"][l]); b2 = f(p["n2_b"][l])
        ff1_w = f(p["ff1_w"][l]); ff1_b = f(p["ff1_b"][l])
        ff1T = (ff1_w * g2[None, :]).T           # [128, 256]
        bff1 = ff1_b + ff1_w @ b2
        ff2T = f(p["ff2_w"][l]).T                # [256, 128]
        wl[l, :, 0:384] = wqkvT
        wl[l, :, 384:512] = woT_half
        wl[l, :, 512:768] = ff1T
        wl[l, :, 768:896] = ff2T[0:128]
        wl[l, :, 896:1024] = ff2T[128:256]
        blb[l, :, 0] = bqkv[0:128]
        blb[l, :, 1] = bqkv[128:256]
        blb[l, :, 2] = bqkv[256:384]
        blb[l, :, 3] = f(p["out_b"][l])
        blb[l, :, 4] = bff1[0:128]
        blb[l, :, 5] = bff1[128:256]
        blb[l, :, 6] = f(p["ff2_b"][l])
    out["wl"] = wl
    out["bl"] = blb
    # q-bias correction lhsT for the attention-score matmul:
    # qb[l][d, head] = bq_d / sqrt(HD) for head = d // HD (zero when bq == 0)
    qb = np.zeros((L, 128, 4), np.float32)
    for l in range(L):
        bq = blb[l, :, 0]
        for d in range(128):
            qb[l, d, d // HD] = bq[d] / math.sqrt(HD)
    out["qb"] = qb

    go = f(p["g_out"]); bo = f(p["bt_out"])
    h1_w = f(p["h1_w"])
    wh = np.zeros((128, 193), np.float32)
    wh[:, 0:128] = (0.5 * h1_w * go[None, :]).T
    wh[:, 128:192] = f(p["h2_w"]).T
    wh[0:64, 192] = f(p["h3_w"])[0]
    out["wh"] = wh
    bh = np.zeros((128, 3), np.float32)
    bh[:, 0] = f(p["h1_b"]) + h1_w @ bo
    bh[0:64, 1] = f(p["h2_b"])
    bh[0, 2] = f(p["h3_b"])[0]
    out["bh"] = bh
    return out


def _static_consts():
    c = {}
    c["C"] = (np.eye(128, dtype=np.float32) - 1.0 / 128.0).astype(np.float32)
    c["Jv"] = np.full((128, 128), 1.0 / 256.0, np.float32)
    c["I"] = np.eye(128, dtype=np.float32)
    c["epsrow"] = np.full((1, 128), EPS / 2.0, np.float32)
    # padded to 36 output rows: rows 4..31 compute zeros so the full
    # [0:36] partition range of the score psum is written (no uninit reads)
    sm = np.zeros((128, 36), np.float32)
    for d in range(128):
        sm[d, d // HD] = 1.0 / math.sqrt(HD)
    c["smask"] = sm
    bc = np.zeros((36, 256), np.float32)
    for d in range(128):
        bc[0 + d // HD, 0 * 128 + d] = 1.0
        bc[32 + d // HD, 1 * 128 + d] = 1.0
    c["bcmask"] = bc
    return c


def r32(ap):
    return ap.bitcast(F32R)


def _mm(nc, out_ps, lhsT, rhs, start, stop):
    """float32r matmul, chunked over the free dim (<=MMC cols per call)."""
    n = rhs.shape[-1]
    nch = (n + MMC - 1) // MMC
    for c in range(nch):
        sl = slice(c * MMC, min((c + 1) * MMC, n))
        nc.tensor.matmul(out_ps[:, sl], lhsT, rhs[:, sl],
                         start=start, stop=stop)


def build_nc(ntiles=NTILES, q_bias=False, interleave=3):
    nc = bacc.Bacc(None, target_bir_lowering=False)
    cst = _static_consts()

    x_d = nc.dram_tensor("x", [BP, 4], F32, kind="ExternalInput")
    wl_d = nc.dram_tensor("wl", [L, 128, 1024], F32, kind="ExternalInput")
    bl_d = nc.dram_tensor("bl", [L, 128, 8], F32, kind="ExternalInput")

    winT_d = nc.dram_tensor("w_inT", [2, 128], F32, kind="ExternalInput")
    eb_d = nc.dram_tensor("eb", [128, 4], F32, kind="ExternalInput")
    wh_d = nc.dram_tensor("wh", [128, 193], F32, kind="ExternalInput")
    bh_d = nc.dram_tensor("bh", [128, 3], F32, kind="ExternalInput")
    o_d = nc.dram_tensor("o", [1, BP], F32, kind="ExternalOutput")

    C_d = nc.inline_tensor(cst["C"], name="Cmat")
    Jv_d = nc.inline_tensor(cst["Jv"], name="Jvmat")
    I_d = nc.inline_tensor(cst["I"], name="Imat")
    eps_d = nc.inline_tensor(cst["epsrow"], name="epsrow")
    sm_d = nc.inline_tensor(cst["smask"], name="smask")
    bc_d = nc.inline_tensor(cst["bcmask"], name="bcmask")

    with tile.TileContext(nc) as tc, ExitStack() as ctx:
        wp = ctx.enter_context(tc.tile_pool(name="weights", bufs=1))
        hp = ctx.enter_context(tc.tile_pool(name="hbuf", bufs=2 * interleave))
        sp = ctx.enter_context(tc.tile_pool(name="scratch", bufs=2))
        yp = ctx.enter_context(tc.tile_pool(name="ybuf", bufs=interleave))
        wsp = ctx.enter_context(tc.tile_pool(name="wstage", bufs=1))
        qp = ctx.enter_context(tc.tile_pool(name="qkv", bufs=interleave))
        psA = ctx.enter_context(tc.tile_pool(name="psA", bufs=4, space="PSUM"))
        psB = ctx.enter_context(tc.tile_pool(name="psB", bufs=4, space="PSUM"))

        def wtile(src, shape, tag):
            t = wp.tile(shape, F32, tag=tag)
            nc.sync.dma_start(t[:], src)
            return t

        def wtile_r(src, shape, tag):
            st = wsp.tile([128, 1024], F32, tag="wstage", name="wstage")
            sv = st[: shape[0], : shape[1]]
            nc.sync.dma_start(sv, src)
            t = wp.tile(shape, F32R, tag=tag)
            nc.scalar.copy(t[:], sv)
            return t

        wl_t = [wtile_r(wl_d[l], [128, 1024], f"wl{l}") for l in range(L)]
        bl_t = [wtile(bl_d[l], [128, 8], f"bl{l}") for l in range(L)]

        winT_t = wtile_r(winT_d[:], [2, 128], "winT")
        eb_t = wtile(eb_d[:], [128, 4], "eb")
        wh_t = wtile_r(wh_d[:], [128, 193], "wh")
        bh_t = wtile(bh_d[:], [128, 3], "bh")
        C_t = wtile_r(C_d[:], [128, 128], "Cm")
        Jv_t = wtile_r(Jv_d[:], [128, 128], "Jv")
        I_t = wtile_r(I_d[:], [128, 128], "Im")
        eps_t = wtile_r(eps_d[:], [1, 128], "epsr")
        sm_t = wtile_r(sm_d[:], [128, 36], "smask")
        sm_bb = wp.tile([128, 36], mybir.dt.bfloat16, tag="smaskb")
        nc.vector.tensor_copy(sm_bb[:], sm_t[:].bitcast(F32))
        bc_t = wtile_r(bc_d[:], [36, 256], "bcm")
        ones_s = wsp.tile([128, 1024], F32, tag="wstage", name="ones_s")
        nc.vector.memset(ones_s[0:1, 0:NT], 1.0)
        ones_t = wp.tile([1, NT], F32R, tag="ones")
        nc.scalar.copy(ones_t[:], ones_s[0:1, 0:NT])

        b_in_ap = eb_t[:, 0:1]
        g_in_ap = eb_t[:, 1:2]
        bias_e0 = eb_t[:, 2:3]
        bias_e1 = eb_t[:, 3:4]

        def layernorm(hsb, tag="", y_dt=F32R, with_eps=False):
            """y = (h - mean)/sqrt(var + eps), [128, NT] sbuf tile.
            Pipelined in two column halves (1-bank psum tiles) so the
            serial square->var->seed->rsqrt->apply chain overlaps.
            with_eps=False skips the eps seed matmul: valid whenever the
            input's per-column variance is >> eps (all LNs except embed)."""
            y = yp.tile([128, NT], y_dt, tag="yln")
            for hf in range(2):
                cs = slice(hf * N, (hf + 1) * N)
                hc_ps = psB.tile([128, N], F32, tag="lnh")
                nc.tensor.matmul(hc_ps[:], C_t[:], hsb[:, cs],
                                 start=True, stop=True)
                sq = sp.tile([128, N], F32R, tag="sq")
                nc.scalar.activation(out=sq[:], in_=hc_ps[:], func=AF.Square,
                                     bias=0.0, scale=1.0)
                vh_ps = psB.tile([128, N], F32, tag="lnh")
                if with_eps:
                    nc.tensor.matmul(vh_ps[:], eps_t[:], ones_t[:, 0:N],
                                     start=True, stop=False)
                    nc.tensor.matmul(vh_ps[:], Jv_t[:], sq[:],
                                     start=False, stop=True)
                else:
                    nc.tensor.matmul(vh_ps[:], Jv_t[:], sq[:],
                                     start=True, stop=True)
                tb = sp.tile([128, N], I32, tag="tbits")
                nc.vector.tensor_scalar(
                    tb[:], vh_ps[:].bitcast(I32), 1, -1,
                    op0=ALU.logical_shift_right, op1=ALU.bitwise_xor)
                nc.vector.tensor_scalar(tb[:], tb[:], SEED_ADD, None,
                                        op0=ALU.add)
                R = sp.tile([128, N], F32, tag="rstd")
                nc.vector._custom_dve(RSQRT_NR2, out=R[:], in0=vh_ps[:],
                                      in1=tb[:].bitcast(F32), s0=1.5)
                nc.vector.tensor_mul(y[:, cs], hc_ps[:], R[:])
            return y

        def emit_embed(it):
            b0 = it * N
            xs0 = sp.tile([2, N], F32, tag="xs0")
            xs1 = sp.tile([2, N], F32, tag="xs1")
            xs = x_d[b0:b0 + N, :]
            nc.sync.dma_start(xs0[:], xs.rearrange("n f -> f n")[0:2, :])
            nc.sync.dma_start(xs1[:], xs.rearrange("n f -> f n")[2:4, :])
            xt0 = sp.tile([2, N], F32R, tag="xt0")
            nc.scalar.copy(xt0[:], xs0[:])
            xt1 = sp.tile([2, N], F32R, tag="xt1")
            nc.scalar.copy(xt1[:], xs1[:])

            h_emb = sp.tile([128, NT], F32R, tag="hemb")
            for hf, xt in ((0, xt0), (1, xt1)):
                pe0 = psA.tile([128, N], F32, tag="mm")
                nc.tensor.matmul(pe0[:], winT_t[:], xt[:],
                                 start=True, stop=True)
                nc.scalar.activation(out=h_emb[:, hf * N:(hf + 1) * N],
                                     in_=pe0[:], func=AF.Identity,
                                     bias=b_in_ap, scale=1.0)

            y_e = layernorm(h_emb, tag="e", y_dt=F32, with_eps=True)
            h = hp.tile([128, NT], F32R, tag="h")
            nc.scalar.activation(out=h[:, 0:N], in_=y_e[:, 0:N],
                                 func=AF.Identity, bias=bias_e0, scale=g_in_ap)
            nc.scalar.activation(out=h[:, N:NT], in_=y_e[:, N:NT],
                                 func=AF.Identity, bias=bias_e1, scale=g_in_ap)
            return h

        def emit_qkv(l, h, y1):
            W = wl_t[l]
            Bb = bl_t[l]
            # k, v, then q into the psum ring: k/v are consumed by fast ACT
            # copies, so the q banks (consumed later by the pr muls) don't
            # stall the in-order PE queue.
            kv_ps = []
            for j in (1, 2):
                for hf in range(2):
                    ps = psA.tile([128, N], F32, tag="mm")
                    nc.tensor.matmul(ps[:], W[:, 128 * j:128 * (j + 1)],
                                     y1[:, hf * N:(hf + 1) * N],
                                     start=True, stop=True)
                    kv_ps.append(ps)
            k_sb = qp.tile([128, NT], mybir.dt.bfloat16, tag="qkv1")
            v_sb = qp.tile([128, NT], mybir.dt.bfloat16, tag="qkv2")
            for hf in range(2):
                nc.scalar.activation(out=k_sb[:, hf * N:(hf + 1) * N],
                                     in_=kv_ps[hf][:], func=AF.Identity,
                                     bias=Bb[:, 1:2], scale=1.0)
                nc.scalar.activation(out=v_sb[:, hf * N:(hf + 1) * N],
                                     in_=kv_ps[2 + hf][:], func=AF.Identity,
                                     bias=Bb[:, 2:3], scale=1.0)
            q_ps = []
            for hf in range(2):
                ps = psA.tile([128, N], F32, tag="mm")
                nc.tensor.matmul(ps[:], W[:, 0:128],
                                 y1[:, hf * N:(hf + 1) * N],
                                 start=True, stop=True)
                q_ps.append(ps)
            dk = sp.tile([128, N], mybir.dt.bfloat16, tag="dk")
            nc.gpsimd.tensor_tensor(dk[:], k_sb[:, 0:N], k_sb[:, N:NT],
                                    op=ALU.subtract)
            # pr_q = (q + bq) * dk, with q read straight from PSUM.  bq is
            # zero for the actual model inputs (checked host-side); when
            # nonzero, q is materialized in SBUF with the bias first.
            pr = [sp.tile([128, N], mybir.dt.bfloat16, tag=f"pr{hf}",
                          name=f"pr{hf}") for hf in range(2)]
            if q_bias:
                q_sb = qp.tile([128, NT], mybir.dt.bfloat16, tag="qkv0")
                for hf in range(2):
                    nc.scalar.activation(out=q_sb[:, hf * N:(hf + 1) * N],
                                         in_=q_ps[hf][:], func=AF.Identity,
                                         bias=Bb[:, 0:1], scale=1.0)
                for hf in range(2):
                    nc.vector.tensor_mul(pr[hf][:],
                                         q_sb[:, hf * N:(hf + 1) * N], dk[:])
            else:
                for hf in range(2):
                    nc.vector.tensor_mul(pr[hf][:], q_ps[hf][:], dk[:])
            return k_sb, v_sb, pr

        def emit_score(l, kvp):
            k_sb, v_sb, pr = kvp
            d_ps = psB.tile([128, N], F32, tag="lnh")
            nc.tensor.matmul(d_ps[0:36, :], sm_bb[:], pr[0][:],
                             start=True, stop=True)
            nc.tensor.matmul(d_ps[32:36, :], sm_bb[:, 0:4], pr[1][:],
                             start=True, stop=True,
                             tile_position=(0, 32))
            T8 = sp.tile([36, N], F32R, tag="T8")
            nc.scalar.activation(out=T8[:], in_=d_ps[0:36, :],
                                 func=AF.Tanh, bias=0.0, scale=0.5)
            dv = sp.tile([128, N], mybir.dt.bfloat16, tag="dv")
            vf = v_sb[:]
            nc.gpsimd.tensor_tensor(dv[:], vf[:, 0:N], vf[:, N:NT],
                                    op=ALU.subtract)
            vs = sp.tile([128, N], F32R, tag="vs")
            nc.gpsimd.tensor_tensor(vs[:], vf[:, 0:N],
                                    vf[:, N:NT], op=ALU.add)
            return T8, dv, vs

        def emit_attnout(l, h, kvp, sc):
            W = wl_t[l]
            Bb = bl_t[l]
            T8, dv, vs = sc
            woT = W[:, 384:512]
            h2t = hp.tile([128, NT], F32R, tag="h")
            us = []
            for hf in range(2):
                tb_ps = psA.tile([128, N], F32, tag="mm")
                nc.tensor.matmul(tb_ps[:], bc_t[:, 128 * hf:128 * (hf + 1)],
                                 T8[:], start=True, stop=True)
                u = sp.tile([128, N], F32R, tag=f"u{hf}", name=f"u{hf}")
                nc.vector.tensor_mul(u[:], tb_ps[:], dv[:])
                us.append(u)
            for hf in range(2):
                sl = slice(hf * N, (hf + 1) * N)
                p1 = psA.tile([128, N], F32, tag="mm")
                nc.tensor.matmul(p1[:], I_t[:], h[:, sl],
                                 start=True, stop=False)
                nc.tensor.matmul(p1[:], woT, vs[:],
                                 start=False, stop=False)
                nc.tensor.matmul(p1[:], woT, us[hf][:],
                                 start=False, stop=True)
                nc.scalar.activation(out=h2t[:, sl], in_=p1[:],
                                     func=AF.Identity,
                                     bias=Bb[:, 3:4], scale=1.0)
            return h2t

        def emit_ff(l, h, y2):
            W = wl_t[l]
            Bb = bl_t[l]
            g0 = sp.tile([128, NT], F32R, tag="g0")
            g1 = sp.tile([128, NT], F32R, tag="g1")
            for gi, g in ((0, g0), (1, g1)):
                for hf in range(2):
                    sl = slice(hf * N, (hf + 1) * N)
                    f = psA.tile([128, N], F32, tag="mm")
                    nc.tensor.matmul(f[:], W[:, 512 + 128 * gi:640 + 128 * gi],
                                     y2[:, sl], start=True, stop=True)
                    nc.scalar.activation(out=g[:, sl], in_=f[:], func=AF.Gelu,
                                         bias=Bb[:, 4 + gi:5 + gi], scale=1.0)
            h3t = hp.tile([128, NT], F32R, tag="h")
            for hf in range(2):
                sl = slice(hf * N, (hf + 1) * N)
                p2 = psA.tile([128, N], F32, tag="mm")
                nc.tensor.matmul(p2[:], I_t[:], h[:, sl],
                                 start=True, stop=False)
                nc.tensor.matmul(p2[:], W[:, 768:896], g0[:, sl],
                                 start=False, stop=False)
                nc.tensor.matmul(p2[:], W[:, 896:1024], g1[:, sl],
                                 start=False, stop=True)
                nc.scalar.activation(out=h3t[:, sl], in_=p2[:],
                                     func=AF.Identity,
                                     bias=Bb[:, 6:7], scale=1.0)
            return h3t

        def emit_head_group(its, hs):
            n = len(its)
            yfs = [layernorm(h, tag="f") for h in hs]
            p3s, p1hs, p4s, p2hs, p5s, ths = [], [], [], [], [], []
            for c in range(n):
                p3 = psB.tile([128, N], F32, tag="lnh", name="p3")
                nc.tensor.matmul(p3[:], wh_t[:, 0:128], yfs[c][:, 0:N],
                                 start=True, stop=False)
                nc.tensor.matmul(p3[:], wh_t[:, 0:128], yfs[c][:, N:NT],
                                 start=False, stop=True)
                p3s.append(p3)
            for c in range(n):
                p1h = sp.tile([128, N], F32R, tag=f"p1h{c}", name="p1h")
                nc.scalar.activation(out=p1h[:], in_=p3s[c][:], func=AF.Gelu,
                                     bias=bh_t[:, 0:1], scale=1.0)
                p1hs.append(p1h)
            for c in range(n):
                p4 = psB.tile([128, N], F32, tag="lnh", name="p4")
                nc.tensor.matmul(p4[0:64, :], wh_t[:, 128:192], p1hs[c][:],
                                 start=True, stop=True)
                p4s.append(p4)
            for c in range(n):
                p2h = sp.tile([64, N], F32R, tag=f"p2h{c}", name="p2h")
                nc.scalar.activation(out=p2h[:], in_=p4s[c][0:64, :],
                                     func=AF.Gelu,
                                     bias=bh_t[0:64, 1:2], scale=1.0)
                p2hs.append(p2h)
            for c in range(n):
                p5 = psB.tile([128, N], F32, tag="lnh", name="p5")
                nc.tensor.matmul(p5[0:1, :], wh_t[0:64, 192:193], p2hs[c][:],
                                 start=True, stop=True)
                p5s.append(p5)
            for c in range(n):
                th = sp.tile([1, N], F32, tag=f"th{c}", name="th")
                nc.scalar.activation(out=th[:], in_=p5s[c][0:1, :],
                                     func=AF.Tanh,
                                     bias=bh_t[0:1, 2:3], scale=1.0)
                ths.append(th)
            for c in range(n):
                b0 = its[c] * N
                res = sp.tile([1, N], F32, tag=f"res{c}", name="res")
                nc.vector.tensor_scalar(res[:], ths[c][:], 3.0, None,
                                        op0=ALU.mult)
                nc.sync.dma_start(o_d[0:1, b0:b0 + N], res[:])

        # software-pipeline groups of super-tiles, phase-rotated: each
        # pipeline stage (LN1 / attn / LN2 / ff) is emitted for ALL chains
        # of the group before the next stage, so every engine's in-order
        # queue alternates between independent chains and one chain's
        # serial LN/attention chain is hidden behind the others' work.
        i = 0
        pending = None
        while i < ntiles:
            grp = list(range(i, min(i + interleave, ntiles)))
            i += len(grp)
            hs = [emit_embed(it) for it in grp]
            if pending is not None:
                # previous group's heads ride behind this group's embeds so
                # their serial tail overlaps the next group's layer-0 work
                emit_head_group(*pending)
            for l in range(L):
                y1s = [layernorm(h, tag="1") for h in hs]
                kvps = [emit_qkv(l, h, y1) for h, y1 in zip(hs, y1s)]
                scs = [emit_score(l, kvp) for kvp in kvps]
                hs = [emit_attnout(l, h, kvp, sc) for h, kvp, sc
                      in zip(hs, kvps, scs)]
                y2s = [layernorm(h, tag="2") for h in hs]
                hs = [emit_ff(l, h, y2) for h, y2 in zip(hs, y2s)]
            pending = (grp, hs)
        emit_head_group(*pending)

    nc.compile()
    return nc


_NC_CACHE = {}


def kernel(**inputs):
    w = _prepare_weights(inputs)
    q_bias = not bool(np.all(w["qb"] == 0.0))
    key = ("nc", q_bias)
    if key not in _NC_CACHE:
        _NC_CACHE[key] = build_nc(q_bias=q_bias)
    nc = _NC_CACHE[key]
    x = np.asarray(inputs["x"], np.float32)
    in_maps = []
    for c in range(NCORES):
        in_maps.append({
            "x": np.ascontiguousarray(x[c * BP:(c + 1) * BP]),
            "wl": w["wl"], "bl": w["bl"], "w_inT": w["w_inT"],
            "eb": w["eb"], "wh": w["wh"], "bh": w["bh"],
        })
    res = run_bass_kernel_spmd(nc, in_maps, core_ids=list(range(NCORES)))
    outs = [res.results[c]["o"].reshape(BP, 1) for c in range(NCORES)]
    return np.concatenate(outs, axis=0).astype(np.float32)


if __name__ == "__main__":
    build_nc(ntiles=1)
    print("build ok")

